# revision 41
# baseline (speedup 1.0000x reference)
"""CrossNetwork (4-layer DCN-v1) Trainium2 Bass kernel.

Math: the reference computes, with x0 = x:
    x_{i+1} = x0 * <x_i, w_i> + b_i + x_i          (i = 0..3)

Every x_i stays of the form  x_i = a_i[row] * x0 + c_i  with
    c_i = sum_{j<i} b_j                (row independent)
    a_{i+1} = a_i * (1 + d_i) + e_i    (per-row scalar recurrence)
    d_i = <x0_row, w_i>                (per-row dots, vs x0 only)
    e_i = <c_i, w_i>                   (scalar constants)
so the full network collapses to:
    out = a_4[:, None] * x0 + (b_0+b_1+b_2+b_3)[None, :]

On-chip per 512-row supertile (layout [128 part, 4 r, 1024 f], row = 4p+r):
  1. PE transposes x chunks -> xT in PSUM
  2. ACT copies xT PSUM->SBUF (rounding to f32r for the dot matmuls)
  3. PE dot-matmuls (f32r, 1 cycle/row) D^T[4, 512] += Wc^T @ xTc
  4. PE transposes D^T -> D[128, 4r, 4i], ACT copies to SBUF
  5. DVE: 1+D, then tensor_tensor_scan implements the a-recurrence
  6. DVE scalar_tensor_tensor: o_t = (x_t * a) + csum, emitted as bf16
  7. store o_t on the sync HWDGE queue, triggers emitted after all the
     loads (HW A/B: SWDGE stores cost ~1-2us more per pass on real
     silicon than the cost model predicts)

The schedule is built so DMA -- the roofline resource -- never idles:
all 8 supertiles stay resident in SBUF so the loads pack back-to-back
on the sync HWDGE queue; the output leaves as bf16 (half the store
bytes, ~0.3% rel err vs the 2e-2 gate; upcast to fp32 on the host);
csum is loaded as one 4KB row and broadcast on-chip via a rank-1 ones
matmul; w/e consts are queued so they slot right after load 0; and a
PE p-state warmup accumulation group ends just after load 0 lands, so
supertile 0 runs at full clock instead of dragging a ~9us ramp-induced
latency into the last stores.  TimelineSim: DMA is gap-free start to
finish; total = 2.0us head + 70.1us DMA + 1.6us drain ceremony.
Sharding: batch dim across 8 cores (4096 rows each), SPMD.
"""

import numpy as np

import concourse.bacc as bacc
import concourse.bass as bass
import concourse.mybir as mybir
import concourse.tile as tile
from concourse.bass_utils import run_bass_kernel_spmd
from concourse.masks import make_identity

N_CORES = 8
B, F, L = 32768, 1024, 4
BS = B // N_CORES          # 4096 rows per core
ST_ROWS = 512              # rows per supertile
N_ST = BS // ST_ROWS       # 8 supertiles per core
R = ST_ROWS // 128         # 4 row-combs per supertile
NCHUNK = F // 128          # 8 feature chunks

FP32 = mybir.dt.float32
BF16 = mybir.dt.bfloat16
ADD = mybir.AluOpType.add
MULT = mybir.AluOpType.mult

_PROGRAM_CACHE = {}


def _build_program(dot_f32r=True, outer_iters=1, store_q="sync_tail"):
    F32R = mybir.dt.float32r
    nc = bacc.Bacc("TRN2")
    x_d = nc.dram_tensor("x", [BS, F], FP32, kind="ExternalInput")
    w_d = nc.dram_tensor("wpack", [128, NCHUNK, L], FP32, kind="ExternalInput")
    e_d = nc.dram_tensor("erep", [128, L], FP32, kind="ExternalInput")
    c_d = nc.dram_tensor("crow", [1, F], FP32, kind="ExternalInput")
    # output leaves the chip as bf16 (upcast to fp32 on the host): same
    # exponent range as fp32, <=0.4% per-element rounding -- far inside
    # the 2e-2 gate -- and it halves the store half of the DMA roofline
    o_d = nc.dram_tensor("out", [BS, F], BF16, kind="ExternalOutput")

    with tile.TileContext(nc) as tc:
        with (
            tc.tile_pool(name="const", bufs=1) as const_pool,
            tc.tile_pool(name="xin", bufs=N_ST) as xpool,
            tc.tile_pool(name="oout", bufs=6) as opool,
            tc.tile_pool(name="xtsb", bufs=3) as xtpool,
            tc.tile_pool(name="small", bufs=2) as spool,
            tc.tile_pool(name="xtps", bufs=3, space="PSUM") as tpsum,
            tc.tile_pool(name="dtps", bufs=2, space="PSUM") as dpsum,
            tc.tile_pool(name="dps", bufs=2, space="PSUM") as dpsum2,
            tc.tile_pool(name="warm", bufs=1, space="PSUM") as warmpool,
        ):
            # supertile 0's load is issued before anything else on the sync
            # queue so the DMA head starts as early as possible
            x0_t = xpool.tile([128, R, F], FP32, tag="x_t")
            nc.sync.dma_start(
                out=x0_t[:],
                in_=x_d[0:ST_ROWS, :].rearrange("(p r) f -> p r f", p=128))
            # csum lands as one 4KB row (sync queue, slotting in right after
            # supertile 0) and is broadcast to all 128 partitions with a
            # rank-1 ones matmul -- saves 1.3us of the DMA-roofline budget
            # vs loading a host-replicated [128, F]
            crow = const_pool.tile([1, F], FP32)
            nc.sync.dma_start(out=crow[:], in_=c_d[:])
            # wpack is third on the sync queue: its descriptors are ready
            # before load 1's, so it lands right after load 0 -- in time
            # for supertile 0's first dot
            wsb_raw = const_pool.tile([128, NCHUNK, L], FP32)
            nc.sync.dma_start(out=wsb_raw[:], in_=w_d[:])
            # e rides the SWDGE queue (prep finishes before load 1 is
            # ready, so it also lands right after load 0)
            esb = const_pool.tile([128, L], FP32)
            nc.gpsimd.dma_start(out=esb[:], in_=e_d[:])
            ones = const_pool.tile([1, 128], FP32)
            nc.gpsimd.memset(ones[:], 1.0)
            ident = const_pool.tile([128, 128], FP32)
            make_identity(nc, ident[:])

            # PE p-state warmup: one long accumulation group (no
            # inter-instruction semaphores, so the cost model's continuous-
            # run clock ramp applies) that ends just AFTER load 0 lands --
            # supertile 0's transposes then find their data-wait already
            # satisfied and start at full clock instead of paying the
            # ~9us cold-start ramp that the load-paced pipeline can never
            # recover before the tail
            warm = warmpool.tile([128, 128], FP32)
            # 7 is the sim-equivalent minimum (7..14 all pack identically);
            # shortest chosen so even at worst-case real LOW-p-state speed
            # (~790ns/inst) the group still ends before load 0 lands and
            # can never delay supertile 0
            n_warm = 7
            for i in range(n_warm):
                nc.tensor.matmul(
                    warm[:], ident[:], ident[:],
                    start=(i == 0), stop=(i == n_warm - 1),
                )

            if dot_f32r:
                # f32r matmul operands must be produced pre-rounded: convert
                # once via ACT (the per-chunk xT copies below do the same)
                wsb = const_pool.tile([128, NCHUNK, L], F32R)
                nc.scalar.copy(wsb[:], wsb_raw[:])
            else:
                wsb = wsb_raw
            csb = const_pool.tile([128, F], FP32)

            def emit_csb():
                # broadcast csum: emitted after supertile 0's chain so the
                # two fp32 matmuls / ACT copies slot into engine slack
                # instead of delaying the first supertile (csb is first
                # consumed by st 0's epilogue, comfortably later)
                for h in range(2):
                    cps = tpsum.tile([128, R * 128], FP32, tag="xt_ps")
                    nc.tensor.matmul(
                        cps[:], ones[:], crow[:, h * 512:(h + 1) * 512],
                        start=True, stop=True,
                    )
                    nc.scalar.copy(csb[:, h * 512:(h + 1) * 512], cps[:])

            pending_stores = []

            def body(x0, after_st0=None):
                for st in range(N_ST):
                    if st == 0 and x0 is not None:
                        x_t = x0
                    else:
                        x_t = xpool.tile([128, R, F], FP32, tag="x_t")
                        src = x_d[st * ST_ROWS:(st + 1) * ST_ROWS, :].rearrange(
                            "(p r) f -> p r f", p=128)
                        nc.sync.dma_start(out=x_t[:], in_=src)

                    # D^T[i, r*128+j] += sum_f w[i,f] * x[row(4j+r), f]
                    dt_ps = dpsum.tile([L, R * 128], FP32)
                    for c in range(NCHUNK):
                        xt_ps = tpsum.tile([128, R * 128], FP32)
                        for r in range(R):
                            nc.tensor.matmul(
                                xt_ps[:, r * 128:(r + 1) * 128],
                                x_t[:, r, c * 128:(c + 1) * 128],
                                ident[:],
                                start=True, stop=True,
                                is_transpose=True,
                            )
                        xt_sb = xtpool.tile(
                            [128, R * 128], F32R if dot_f32r else FP32)
                        nc.scalar.copy(xt_sb[:], xt_ps[:])
                        nc.tensor.matmul(
                            dt_ps[:],
                            wsb[:, c, :],
                            xt_sb[:],
                            start=(c == 0), stop=(c == NCHUNK - 1),
                        )

                    if st == 0 and after_st0 is not None:
                        after_st0()

                    dt_sb = spool.tile([L, R * 128], FP32, tag="dt_sb")
                    nc.scalar.copy(dt_sb[:], dt_ps[:])

                    # transpose D^T -> D [128 j, r, i]
                    d_ps = dpsum2.tile([128, R, L], FP32)
                    for r in range(R):
                        nc.tensor.matmul(
                            d_ps[:, r, :],
                            dt_sb[:, r * 128:(r + 1) * 128],
                            ident[:L, :L],
                            start=True, stop=True,
                        )
                    d_sb = spool.tile([128, R, L], FP32, tag="d_sb")
                    nc.scalar.copy(d_sb[:], d_ps[:])

                    # a-recurrence: state=1; state = ((1+d_i)*state) + e_i
                    pd = spool.tile([128, R, L], FP32, tag="pd")
                    nc.vector.tensor_scalar_add(pd[:], d_sb[:], 1.0)
                    sc = spool.tile([128, R, L], FP32, tag="sc")
                    for r in range(R):
                        nc.vector.tensor_tensor_scan(
                            sc[:, r, :], pd[:, r, :], esb[:],
                            1.0, MULT, ADD,
                        )

                    # epilogue: out = (x * a) + csum, downconverted to bf16
                    # by the DVE pass itself
                    o_t = opool.tile([128, R, F], BF16)
                    for r in range(R):
                        nc.vector.scalar_tensor_tensor(
                            o_t[:, r, :], x_t[:, r, :], sc[:, r, L - 1:L],
                            csb[:], MULT, ADD,
                        )
                    dst = o_d[st * ST_ROWS:(st + 1) * ST_ROWS, :].rearrange(
                        "(p r) f -> p r f", p=128)
                    # store_q: "gpsimd" = SWDGE, "scalar" = ACT HWDGE queue,
                    # "sync_tail" = SP HWDGE queue with all store triggers
                    # emitted after the loads (no head-of-line blocking)
                    if store_q == "sync_tail":
                        pending_stores.append((dst, o_t))
                    else:
                        getattr(nc, store_q).dma_start(out=dst, in_=o_t[:])
                if store_q == "sync_tail":
                    for dst, o in pending_stores:
                        nc.sync.dma_start(out=dst, in_=o[:])
                    pending_stores.clear()

            if outer_iters == 1:
                body(x0_t, after_st0=emit_csb)
            else:
                # chained timing build: csb built once up front; every
                # iteration re-loads st 0
                emit_csb()
                with tc.For_i(0, outer_iters):
                    body(None)
    nc.compile()
    return nc


def _host_prep(Ws, Bs):
    Ws = np.asarray(Ws, dtype=np.float32)
    Bs = np.asarray(Bs, dtype=np.float32)
    # wpack[p, c, i] = Ws[i, c*128 + p]
    wpack = np.ascontiguousarray(
        Ws.reshape(L, NCHUNK, 128).transpose(2, 1, 0))
    csum = np.zeros(F, np.float32)
    e = np.zeros(L, np.float32)
    for i in range(L):
        e[i] = np.float32(csum @ Ws[i])
        csum = (csum + Bs[i]).astype(np.float32)
    erep = np.broadcast_to(e, (128, L)).copy()
    crow = csum.reshape(1, F).copy()
    return wpack, erep, crow


def _get_program(**opts):
    key = tuple(sorted(opts.items()))
    if key not in _PROGRAM_CACHE:
        _PROGRAM_CACHE[key] = _build_program(**opts)
    return _PROGRAM_CACHE[key]


def _in_maps(x, Ws, Bs):
    x = np.asarray(x, dtype=np.float32)
    wpack, erep, crow = _host_prep(Ws, Bs)
    return [
        {
            "x": np.ascontiguousarray(x[k * BS:(k + 1) * BS]),
            "wpack": wpack,
            "erep": erep,
            "crow": crow,
        }
        for k in range(N_CORES)
    ]


def _run(x, Ws, Bs, trace=False, trace_kwargs=None, **opts):
    nc = _get_program(**opts)
    in_maps = _in_maps(x, Ws, Bs)
    res = run_bass_kernel_spmd(
        nc, in_maps, list(range(N_CORES)),
        trace=trace, **(trace_kwargs or {}),
    )
    out = np.concatenate(
        [np.asarray(res.results[k]["out"]) for k in range(N_CORES)], axis=0
    ).astype(np.float32)
    return out, res


def kernel(x, Ws, Bs):
    out, _ = _run(x, Ws, Bs, trace=False)
    return out


# revision 42
# speedup vs baseline: 1.2891x; 1.2891x over previous
"""CrossNetwork (4-layer DCN-v1) Trainium2 Bass kernel.

Math: the reference computes, with x0 = x:
    x_{i+1} = x0 * <x_i, w_i> + b_i + x_i          (i = 0..3)

Every x_i stays of the form  x_i = a_i[row] * x0 + c_i  with
    c_i = sum_{j<i} b_j                (row independent)
    a_{i+1} = a_i * (1 + d_i) + e_i    (per-row scalar recurrence)
    d_i = <x0_row, w_i>                (per-row dots, vs x0 only)
    e_i = <c_i, w_i>                   (scalar constants)
so the full network collapses to:
    out = a_4[:, None] * x0 + (b_0+b_1+b_2+b_3)[None, :]

On-chip per 512-row supertile (layout [128 part, 4 r, 1024 f], row = 4p+r):
  1. PE transposes x chunks -> xT in PSUM
  2. ACT copies xT PSUM->SBUF (rounding to f32r for the dot matmuls)
  3. PE dot-matmuls (f32r, 1 cycle/row) D^T[4, 512] += Wc^T @ xTc
  4. PE transposes D^T -> D[128, 4r, 4i], ACT copies to SBUF
  5. DVE: 1+D, then tensor_tensor_scan implements the a-recurrence
  6. DVE scalar_tensor_tensor: o_t = (x_t * a) + csum, emitted as bf16
  7. store o_t on the sync HWDGE queue, triggers emitted after all the
     loads (HW A/B: SWDGE stores cost ~1-2us more per pass on real
     silicon than the cost model predicts)

The schedule is built so DMA -- the roofline resource -- never idles:
all 8 supertiles stay resident in SBUF so the loads pack back-to-back
on the sync HWDGE queue; the output leaves as bf16 (half the store
bytes, ~0.3% rel err vs the 2e-2 gate; upcast to fp32 on the host);
csum is loaded as one 4KB row and broadcast on-chip via a rank-1 ones
matmul; w/e consts are queued so they slot right after load 0; and a
PE p-state warmup accumulation group ends just after load 0 lands, so
supertile 0 runs at full clock instead of dragging a ~9us ramp-induced
latency into the last stores.  TimelineSim: DMA is gap-free start to
finish; total = 2.0us head + 70.1us DMA + 1.6us drain ceremony.
Sharding: batch dim across 8 cores (4096 rows each), SPMD.
"""

import numpy as np

import concourse.bacc as bacc
import concourse.bass as bass
import concourse.mybir as mybir
import concourse.tile as tile
from concourse.bass_utils import run_bass_kernel_spmd
from concourse.masks import make_identity

N_CORES = 8
B, F, L = 32768, 1024, 4
BS = B // N_CORES          # 4096 rows per core
ST_ROWS = 512              # rows per supertile
N_ST = BS // ST_ROWS       # 8 supertiles per core
R = ST_ROWS // 128         # 4 row-combs per supertile
NCHUNK = F // 128          # 8 feature chunks

FP32 = mybir.dt.float32
BF16 = mybir.dt.bfloat16
ADD = mybir.AluOpType.add
MULT = mybir.AluOpType.mult

_PROGRAM_CACHE = {}


def _build_program(dot_f32r=True, outer_iters=1, store_q="sync_tail"):
    F32R = mybir.dt.float32r
    nc = bacc.Bacc("TRN2")
    x_d = nc.dram_tensor("x", [BS, F], FP32, kind="ExternalInput")
    w_d = nc.dram_tensor("wpack", [128, NCHUNK, L], FP32, kind="ExternalInput")
    e_d = nc.dram_tensor("erep", [128, L], FP32, kind="ExternalInput")
    c_d = nc.dram_tensor("crow", [1, F], FP32, kind="ExternalInput")
    # output leaves the chip as bf16 (upcast to fp32 on the host): same
    # exponent range as fp32, <=0.4% per-element rounding -- far inside
    # the 2e-2 gate -- and it halves the store half of the DMA roofline
    o_d = nc.dram_tensor("out", [BS, F], BF16, kind="ExternalOutput")

    with tile.TileContext(nc) as tc:
        with (
            tc.tile_pool(name="const", bufs=1) as const_pool,
            tc.tile_pool(name="xin", bufs=N_ST) as xpool,
            # 7 output bufs = the SBUF maximum (8 overflows): only the last
            # supertile's epilogue can ever wait on a store completion,
            # hedging against real store-drain latency the model undercounts
            tc.tile_pool(name="oout", bufs=7) as opool,
            tc.tile_pool(name="xtsb", bufs=3) as xtpool,
            tc.tile_pool(name="small", bufs=2) as spool,
            tc.tile_pool(name="xtps", bufs=3, space="PSUM") as tpsum,
            tc.tile_pool(name="dtps", bufs=2, space="PSUM") as dpsum,
            tc.tile_pool(name="dps", bufs=2, space="PSUM") as dpsum2,
            tc.tile_pool(name="warm", bufs=1, space="PSUM") as warmpool,
        ):
            # supertile 0's load is issued before anything else on the sync
            # queue so the DMA head starts as early as possible
            x0_t = xpool.tile([128, R, F], FP32, tag="x_t")
            nc.sync.dma_start(
                out=x0_t[:],
                in_=x_d[0:ST_ROWS, :].rearrange("(p r) f -> p r f", p=128))
            # csum lands as one 4KB row (sync queue, slotting in right after
            # supertile 0) and is broadcast to all 128 partitions with a
            # rank-1 ones matmul -- saves 1.3us of the DMA-roofline budget
            # vs loading a host-replicated [128, F]
            crow = const_pool.tile([1, F], FP32)
            nc.sync.dma_start(out=crow[:], in_=c_d[:])
            # wpack is third on the sync queue: its descriptors are ready
            # before load 1's, so it lands right after load 0 -- in time
            # for supertile 0's first dot
            wsb_raw = const_pool.tile([128, NCHUNK, L], FP32)
            nc.sync.dma_start(out=wsb_raw[:], in_=w_d[:])
            # e rides the SWDGE queue (prep finishes before load 1 is
            # ready, so it also lands right after load 0)
            esb = const_pool.tile([128, L], FP32)
            nc.gpsimd.dma_start(out=esb[:], in_=e_d[:])
            ones = const_pool.tile([1, 128], FP32)
            nc.gpsimd.memset(ones[:], 1.0)
            ident = const_pool.tile([128, 128], FP32)
            make_identity(nc, ident[:])

            # PE p-state warmup: one long accumulation group (no
            # inter-instruction semaphores, so the cost model's continuous-
            # run clock ramp applies) that ends just AFTER load 0 lands --
            # supertile 0's transposes then find their data-wait already
            # satisfied and start at full clock instead of paying the
            # ~9us cold-start ramp that the load-paced pipeline can never
            # recover before the tail
            warm = warmpool.tile([128, 128], FP32)
            # 7 is the sim-equivalent minimum (7..14 all pack identically);
            # shortest chosen so even at worst-case real LOW-p-state speed
            # (~790ns/inst) the group still ends before load 0 lands and
            # can never delay supertile 0
            n_warm = 7
            for i in range(n_warm):
                nc.tensor.matmul(
                    warm[:], ident[:], ident[:],
                    start=(i == 0), stop=(i == n_warm - 1),
                )

            if dot_f32r:
                # f32r matmul operands must be produced pre-rounded: convert
                # once via ACT (the per-chunk xT copies below do the same)
                wsb = const_pool.tile([128, NCHUNK, L], F32R)
                nc.scalar.copy(wsb[:], wsb_raw[:])
            else:
                wsb = wsb_raw
            csb = const_pool.tile([128, F], FP32)

            def emit_csb():
                # broadcast csum: emitted after supertile 0's chain so the
                # two fp32 matmuls / ACT copies slot into engine slack
                # instead of delaying the first supertile (csb is first
                # consumed by st 0's epilogue, comfortably later)
                for h in range(2):
                    cps = tpsum.tile([128, R * 128], FP32, tag="xt_ps")
                    nc.tensor.matmul(
                        cps[:], ones[:], crow[:, h * 512:(h + 1) * 512],
                        start=True, stop=True,
                    )
                    nc.scalar.copy(csb[:, h * 512:(h + 1) * 512], cps[:])

            pending_stores = []

            def body(x0, after_st0=None):
                for st in range(N_ST):
                    if st == 0 and x0 is not None:
                        x_t = x0
                    else:
                        x_t = xpool.tile([128, R, F], FP32, tag="x_t")
                        src = x_d[st * ST_ROWS:(st + 1) * ST_ROWS, :].rearrange(
                            "(p r) f -> p r f", p=128)
                        nc.sync.dma_start(out=x_t[:], in_=src)

                    # D^T[i, r*128+j] += sum_f w[i,f] * x[row(4j+r), f]
                    dt_ps = dpsum.tile([L, R * 128], FP32)
                    for c in range(NCHUNK):
                        xt_ps = tpsum.tile([128, R * 128], FP32)
                        for r in range(R):
                            nc.tensor.matmul(
                                xt_ps[:, r * 128:(r + 1) * 128],
                                x_t[:, r, c * 128:(c + 1) * 128],
                                ident[:],
                                start=True, stop=True,
                                is_transpose=True,
                            )
                        xt_sb = xtpool.tile(
                            [128, R * 128], F32R if dot_f32r else FP32)
                        nc.scalar.copy(xt_sb[:], xt_ps[:])
                        nc.tensor.matmul(
                            dt_ps[:],
                            wsb[:, c, :],
                            xt_sb[:],
                            start=(c == 0), stop=(c == NCHUNK - 1),
                        )

                    if st == 0 and after_st0 is not None:
                        after_st0()

                    dt_sb = spool.tile([L, R * 128], FP32, tag="dt_sb")
                    nc.scalar.copy(dt_sb[:], dt_ps[:])

                    # transpose D^T -> D [128 j, r, i]
                    d_ps = dpsum2.tile([128, R, L], FP32)
                    for r in range(R):
                        nc.tensor.matmul(
                            d_ps[:, r, :],
                            dt_sb[:, r * 128:(r + 1) * 128],
                            ident[:L, :L],
                            start=True, stop=True,
                        )
                    d_sb = spool.tile([128, R, L], FP32, tag="d_sb")
                    nc.scalar.copy(d_sb[:], d_ps[:])

                    # a-recurrence: state=1; state = ((1+d_i)*state) + e_i
                    pd = spool.tile([128, R, L], FP32, tag="pd")
                    nc.vector.tensor_scalar_add(pd[:], d_sb[:], 1.0)
                    sc = spool.tile([128, R, L], FP32, tag="sc")
                    for r in range(R):
                        nc.vector.tensor_tensor_scan(
                            sc[:, r, :], pd[:, r, :], esb[:],
                            1.0, MULT, ADD,
                        )

                    # epilogue: out = (x * a) + csum, downconverted to bf16
                    # by the DVE pass itself
                    o_t = opool.tile([128, R, F], BF16)
                    for r in range(R):
                        nc.vector.scalar_tensor_tensor(
                            o_t[:, r, :], x_t[:, r, :], sc[:, r, L - 1:L],
                            csb[:], MULT, ADD,
                        )
                    dst = o_d[st * ST_ROWS:(st + 1) * ST_ROWS, :].rearrange(
                        "(p r) f -> p r f", p=128)
                    # store_q: "gpsimd" = SWDGE, "scalar" = ACT HWDGE queue,
                    # "sync_tail" = SP HWDGE queue with all store triggers
                    # emitted after the loads (no head-of-line blocking)
                    if store_q == "sync_tail":
                        pending_stores.append((dst, o_t))
                    else:
                        getattr(nc, store_q).dma_start(out=dst, in_=o_t[:])
                if store_q == "sync_tail":
                    for dst, o in pending_stores:
                        nc.sync.dma_start(out=dst, in_=o[:])
                    pending_stores.clear()

            if outer_iters == 1:
                body(x0_t, after_st0=emit_csb)
            else:
                # chained timing build: csb built once up front; every
                # iteration re-loads st 0
                emit_csb()
                with tc.For_i(0, outer_iters):
                    body(None)
    nc.compile()
    return nc


def _host_prep(Ws, Bs):
    Ws = np.asarray(Ws, dtype=np.float32)
    Bs = np.asarray(Bs, dtype=np.float32)
    # wpack[p, c, i] = Ws[i, c*128 + p]
    wpack = np.ascontiguousarray(
        Ws.reshape(L, NCHUNK, 128).transpose(2, 1, 0))
    csum = np.zeros(F, np.float32)
    e = np.zeros(L, np.float32)
    for i in range(L):
        e[i] = np.float32(csum @ Ws[i])
        csum = (csum + Bs[i]).astype(np.float32)
    erep = np.broadcast_to(e, (128, L)).copy()
    crow = csum.reshape(1, F).copy()
    return wpack, erep, crow


def _get_program(**opts):
    key = tuple(sorted(opts.items()))
    if key not in _PROGRAM_CACHE:
        _PROGRAM_CACHE[key] = _build_program(**opts)
    return _PROGRAM_CACHE[key]


def _in_maps(x, Ws, Bs):
    x = np.asarray(x, dtype=np.float32)
    wpack, erep, crow = _host_prep(Ws, Bs)
    return [
        {
            "x": np.ascontiguousarray(x[k * BS:(k + 1) * BS]),
            "wpack": wpack,
            "erep": erep,
            "crow": crow,
        }
        for k in range(N_CORES)
    ]


def _run(x, Ws, Bs, trace=False, trace_kwargs=None, **opts):
    nc = _get_program(**opts)
    in_maps = _in_maps(x, Ws, Bs)
    res = run_bass_kernel_spmd(
        nc, in_maps, list(range(N_CORES)),
        trace=trace, **(trace_kwargs or {}),
    )
    out = np.concatenate(
        [np.asarray(res.results[k]["out"]) for k in range(N_CORES)], axis=0
    ).astype(np.float32)
    return out, res


def kernel(x, Ws, Bs):
    out, _ = _run(x, Ws, Bs, trace=False)
    return out
